# revision 15
# baseline (speedup 1.0000x reference)
"""Trainium2 Bass kernel for block-local MultiHeadAttention + output projection.

Reference computation (per batch b):
  Q = x @ Wq.T ; K = x @ Wk.T ; V = x @ Wv.T          x: [B, S=8192, 64]
  reshape to [B, G=512, H=16, 64] (groups of 16 consecutive tokens)
  E[g,h,k] = Q[g,h,:] . K[g,k,:]                      (16x16 block-diag attention)
  A = softmax(E / 32, axis=k)
  O[g,h,:] = sum_k A[g,h,k] V[g,k,:]
  out2[b, r, gm*64+d] = O[g=(gq,gm), h, d]  with r = h*32+gq
  y = out2 @ Wo.T + bo                                y: [B, 512, 1024]

Kernel strategy (data-parallel over batch, 4 batches/core on 8 cores):
  - host prep: M^T = Wk^T Wq (so E[h,k] = X_h . Z_k with Z = X M^T, skipping
    Q,K), WoV^T = (Wo @ blockdiag(Wv))^T in fc-ready chunk layout (skips V),
    x cast to bf16 and pre-marshalled into the two SBUF layouts the PE
    consumes (XT feature-major, XPP slab-token-major), plus constants
  - ZT = M X^T on device (row-tiled concurrent matmul pairs)
  - per gm-PAIR (c = gm//2): 8 E^T matmuls (row-tiled by q-parity into
    parity-split banks of one contiguous [128,1024] eps tile), ONE exp,
    ONE mask mul, then per gm 4 U^T matmuls + 1 den ones-matmul into a
    shared [128,1024] ud tile (U cols 0-511 q-order, den cols 512+ aex-order)
  - normalization: reciprocal_approx_fast(den) once per pair; the mul's
    in1 AP permutes aex->q order on the fly
  - fc: y-tile = (out2^T-tile stationary) @ WoV^T streaming; bias added via
    tensor_add during the PSUM eviction; y stored bf16, host casts f32
"""

import numpy as np
from contextlib import ExitStack

import concourse.bass as bass
import concourse.bacc as bacc
import concourse.mybir as mybir
import concourse.tile as tile

N_CORES = 8
B_GLOB = 32
B_LOC = B_GLOB // N_CORES   # 4 batches per core
SB = 8192                   # tokens per batch
D = 64                      # head dim
NG = 16                     # gm values (heads)
NQ = 4                      # gq octs per batch
NJ = 8                      # groups per slab
NH = 16                     # tokens per group
E = 1024
RB = 512                    # out2 rows per batch
NSLAB = B_LOC * NG * NQ     # 256 slabs per core
TOK = B_LOC * SB            # 32768 tokens per core

BF = mybir.dt.bfloat16
F32 = mybir.dt.float32
AF = mybir.ActivationFunctionType


def slab_xt_ap(T, b, gm, q):
    """[64@(q%2), 128] contiguous view of slab (b,gm,q) in XT/ZT layout:
    col = (sidx//2)*128 + k*8 + j, rows (q%2)*64 + d."""
    sidx = (b * 16 + gm) * 4 + q
    half = (sidx % 2) * 64
    pair = sidx // 2
    return T[half:half + 64, pair * 128:(pair + 1) * 128]


def emit_body(ctx, tc, ins, outs, dbg, stage=99):
    nc = tc.nc
    xt, xpp, mt, wovt, mask2, ones64, bias128 = ins
    y = outs["y"]

    # ---------------- persistent tensors ----------------
    pp = ctx.enter_context(tc.tile_pool(name="persist", bufs=1))
    XT = pp.tile([128, 8 * NH * 128], BF, tag="XT")     # [(q%2)*64+d, (bQ|k|j,gm)]
    ZT = pp.tile([128, 8 * NH * 128], BF, tag="ZT")
    XPP = pp.tile([128, NSLAB * D], BF, tag="XPP")      # [k*8+j, (sidx|d)]
    WOVT = pp.tile([128, 8 * E], BF, tag="WOVT")        # WoV^T chunks
    OUT2T = pp.tile([128, B_LOC * 8 * RB], BF, tag="OUT2T")
    MASK2 = pp.tile([128, 1024], BF, tag="MASK2")       # kron(ones16, eye8) x8
    ONES64 = pp.tile([128, D], BF, tag="ONES64")
    MT = pp.tile([128, D], BF, tag="MT")                # M^T dup on both halves
    BIAS = pp.tile([128, E], F32, tag="BIAS")           # bias replicated 128 rows

    # ---------------- input loads, ordered by first use --------
    # queues are FIFO per engine: batch-0 x chunks and the small consts go
    # first; WOVT/BIAS (only needed by fc, ~60us in) go last
    if stage < 1:
        return
    CB = 8 * NH * 128 // B_LOC      # XT/XPP cols per batch (4096)
    # batch 0 lands in fine-grained chunks so ZT/attention start ASAP
    nc.sync.dma_start(XT[:, 0:1024], xt[:, 0:1024])
    nc.sync.dma_start(MT[:], mt)
    nc.sync.dma_start(XPP[:, 0:1024], xpp[:, 0:1024])
    nc.sync.dma_start(MASK2[:], mask2)
    nc.sync.dma_start(ONES64[:], ones64)
    for s in range(1, 4):
        nc.sync.dma_start(XT[:, s * 1024:(s + 1) * 1024],
                          xt[:, s * 1024:(s + 1) * 1024])
        nc.sync.dma_start(XPP[:, s * 1024:(s + 1) * 1024],
                          xpp[:, s * 1024:(s + 1) * 1024])
    for b in range(1, B_LOC):
        nc.sync.dma_start(XT[:, b * CB:(b + 1) * CB], xt[:, b * CB:(b + 1) * CB])
        nc.sync.dma_start(XPP[:, b * CB:(b + 1) * CB], xpp[:, b * CB:(b + 1) * CB])
    nc.sync.dma_start(BIAS[:], bias128)
    nc.sync.dma_start(WOVT[:], wovt)

    if stage < 2:
        return
    big_pool = ctx.enter_context(tc.tile_pool(name="bigps", bufs=2, space="PSUM"))
    eps_pool = fc_pool = big_pool
    ud_pool = ctx.enter_context(tc.tile_pool(name="ud", bufs=2, space="PSUM"))
    zt_pool = ud_pool               # ZT phase borrows the ud slots
    aex_pool = ctx.enter_context(tc.tile_pool(name="aex", bufs=4))
    am_pool = ctx.enter_context(tc.tile_pool(name="am", bufs=4))
    rden_pool = ctx.enter_context(tc.tile_pool(name="rden", bufs=3))
    fout_pool = ctx.enter_context(tc.tile_pool(name="fout", bufs=2))

    def emit_zt(b, rb):
        """One 1024-col ZT chunk for batch b (borrows a ud-pool slot)."""
        r = b * (CB // 1024) + rb
        zfull = zt_pool.tile([128, 1024], F32, tag="ud")
        for hz in range(2):
            zps = zfull[:, hz * 512:(hz + 1) * 512]
            cl = (r * 2 + hz) * 512
            nc.tensor.matmul(zps[0:64, :], MT[0:64, :],
                             XT[0:64, cl:cl + 512], start=True, stop=True)
            nc.tensor.matmul(zps[64:128, :], MT[64:128, :],
                             XT[64:128, cl:cl + 512], start=True, stop=True,
                             tile_position=(64, 64))
        nc.any.tensor_copy(ZT[:, r * 1024:(r + 1) * 1024], zfull[:])

    for b in range(B_LOC):
        # ZT for batch 0 up front; later batches interleave into the
        # previous batch's attention loop (below)
        if stage < 3:
            continue
        if b == 0:
            for rb in range(CB // 1024):
                emit_zt(0, rb)

        # ---------------- attention, per gm pair ----------------
        if stage < 4:
            continue
        for c in range(8):
            if c % 2 == 1 and b + 1 < B_LOC and stage >= 3:
                emit_zt(b + 1, c // 2)
            # E^T matmuls for both gms of the pair, row-tiled by q-parity;
            # eps col = (q%2)*512 + (gm%2)*256 + (q//2)*128  (parity-split banks)
            eps = eps_pool.tile([128, 1024], F32, tag="bigps")
            for gmh in range(2):
                gm = c * 2 + gmh
                for q in range(NQ):
                    col = (q % 2) * 512 + gmh * 256 + (q // 2) * 128
                    nc.tensor.matmul(
                        eps[:, col:col + 128],
                        slab_xt_ap(ZT, b, gm, q),
                        slab_xt_ap(XT, b, gm, q),
                        start=True, stop=True,
                        tile_position=((q % 2) * 64, 0),
                    )
            if stage < 4.2:
                continue
            aex = aex_pool.tile([128, 1024], BF, tag="aex")
            nc.scalar.activation(aex[:], eps[:], AF.Exp, scale=1.0 / 32.0)
            if stage < 4.4:
                continue
            am = am_pool.tile([128, 1024], BF, tag="am")
            nc.vector.tensor_mul(am[:], aex[:], MASK2[:])
            if stage < 4.6:
                continue
            ud = ud_pool.tile([128, 1024], F32, tag="ud")
            amr = am[:].rearrange("p (par gmh2 qh hj) -> gmh2 p par qh hj",
                                  par=2, gmh2=2, qh=2)
            for gmh in range(2):
                gm = c * 2 + gmh
                pb = gmh * 64
                for q in range(NQ):
                    sidx = (b * 16 + gm) * 4 + q
                    acol = (q % 2) * 512 + gmh * 256 + (q // 2) * 128
                    nc.tensor.matmul(ud[pb:pb + 64, q * 128:(q + 1) * 128],
                                     XPP[:, sidx * D:(sidx + 1) * D],
                                     am[:, acol:acol + 128],
                                     start=True, stop=True, tile_position=(0, pb))
                nc.tensor.matmul(ud[pb:pb + 64, 512:1024], ONES64[:], amr[gmh],
                                 start=True, stop=True, tile_position=(0, pb))
            if stage < 4.8:
                continue
            # rden in aex order [par qh hj]; the mul's in1 AP maps to q order
            rden = rden_pool.tile([128, 512], F32, tag="rden")
            nc.vector.reciprocal_approx_fast(rden[:], ud[:, 512:1024])
            rden_q = rden[:].rearrange("p (Y X hj) -> p X Y hj",
                                       Y=2, X=2, hj=128)
            sec = (b * 8 + c) * 512
            out_ap = OUT2T[:, sec:sec + 512].rearrange(
                "p (h q2 j) -> p q2 h j", h=NH, q2=NQ, j=NJ)
            nc.vector.tensor_mul(out_ap, ud[:, 0:512], rden_q)

        # ---------------- fc for this batch ----------------
        if stage < 6:
            continue
        for rt in range(4):
            fo = fout_pool.tile([128, E], BF, tag="fout")
            for halfe in range(2):
                fps_full = fc_pool.tile([128, 1024], F32, tag="bigps")
                fps = fps_full[:, 0:512]
                for cc in range(8):
                    sec = (b * 8 + cc) * 512
                    nc.tensor.matmul(
                        fps[:],
                        OUT2T[:, sec + rt * 128: sec + (rt + 1) * 128],
                        WOVT[:, cc * E + halfe * 512: cc * E + halfe * 512 + 512],
                        start=(cc == 0), stop=(cc == 7),
                    )
                nc.any.tensor_add(fo[:, halfe * 512:(halfe + 1) * 512],
                                  fps[:],
                                  BIAS[:, halfe * 512:(halfe + 1) * 512])
            row = b * RB + rt * 128
            nc.sync.dma_start(y[row:row + 128, :], fo[:])

    # ---------------- debug dumps ----------------
    for name, T in (("xt", XT), ("zt", ZT), ("xpp", XPP), ("out2t", OUT2T)):
        if name in dbg:
            nc.sync.dma_start(dbg[name], T[:])


def build(reps=1, debug=(), stage=99):
    nc = bacc.Bacc("TRN2", target_bir_lowering=False, debug=False,
                   num_devices=N_CORES)
    xt = nc.dram_tensor("xt", [128, 8 * NH * 128], BF, kind="ExternalInput").ap()
    xpp = nc.dram_tensor("xpp", [128, NSLAB * D], BF, kind="ExternalInput").ap()
    mt = nc.dram_tensor("mt", [128, D], BF, kind="ExternalInput").ap()
    wovt = nc.dram_tensor("wovt", [128, 8 * E], BF, kind="ExternalInput").ap()
    mask2 = nc.dram_tensor("mask2", [128, 1024], BF, kind="ExternalInput").ap()
    ones64 = nc.dram_tensor("ones64", [128, D], BF, kind="ExternalInput").ap()
    bias128 = nc.dram_tensor("bias128", [128, E], F32, kind="ExternalInput").ap()
    y = nc.dram_tensor("y", [B_LOC * RB, E], BF, kind="ExternalOutput").ap()
    dbg = {}
    for name, shape, dt in [
        ("xt", [128, 8 * NH * 128], BF),
        ("zt", [128, 8 * NH * 128], BF),
        ("xpp", [128, NSLAB * D], BF),
        ("out2t", [128, B_LOC * 8 * RB], BF),
    ]:
        if name in debug:
            dbg[name] = nc.dram_tensor(name, shape, dt, kind="ExternalOutput").ap()

    ins = (xt, xpp, mt, wovt, mask2, ones64, bias128)
    outs = {"y": y}
    with tile.TileContext(nc) as tc:
        with ExitStack() as ctx:
            if reps > 1:
                with tc.For_i(0, reps, 1):
                    emit_body(ctx, tc, ins, outs, dbg, stage=stage)
            else:
                emit_body(ctx, tc, ins, outs, dbg, stage=stage)
    nc.compile()
    return nc


def host_inputs(x, Wq, Wk, Wv, Wo, bo):
    """Host-side weight prep + x marshalling shared by kernel() and tests."""
    import ml_dtypes
    bf16 = ml_dtypes.bfloat16
    x = np.asarray(x, np.float32)
    Wq = np.asarray(Wq, np.float32)
    Wk = np.asarray(Wk, np.float32)
    Wv = np.asarray(Wv, np.float32)
    Wo = np.asarray(Wo, np.float32)
    bo = np.asarray(bo, np.float32)

    MTh = Wk.T @ Wq                                    # M^T, M = Wq.T @ Wk
    mt = np.concatenate([MTh, MTh], axis=0).astype(bf16)          # [128, 64]
    # WoV[e, g*64+d] = sum_v Wo[e, g*64+v] Wv[v, d]
    WoV = np.matmul(Wo.reshape(E, NG, D), Wv).reshape(E, E)
    WoVT = np.ascontiguousarray(WoV.T)                 # [feature, e_out]
    wovt = np.ascontiguousarray(
        WoVT.reshape(8, 128, E).transpose(1, 0, 2).reshape(128, 8 * E)
    ).astype(bf16)
    blk = np.kron(np.ones((16, 16), np.float32), np.eye(8, dtype=np.float32))
    mask2 = np.tile(blk, (1, 8)).astype(bf16)
    ones64 = np.ones((128, D), dtype=bf16)
    bias128 = np.ascontiguousarray(np.tile(bo[None, :], (128, 1)),
                                   dtype=np.float32)
    xbf = x.astype(bf16)
    shared = {"mt": mt, "wovt": wovt, "mask2": mask2, "ones64": ones64,
              "bias128": bias128}
    in_maps = []
    for core in range(N_CORES):
        xs = xbf[core * B_LOC:(core + 1) * B_LOC]
        # token t = ((q*8+j)*16+gm)*16+k, q = qh*2+qp
        x5 = xs.reshape(B_LOC, 2, 2, NJ, NG, NH, D)   # [b,qh,qp,j,gm,k,d]
        xt = np.ascontiguousarray(
            x5.transpose(2, 6, 0, 4, 1, 5, 3)          # [qp,d,b,gm,qh,k,j]
        ).reshape(128, 8 * NH * 128)
        xpp = np.ascontiguousarray(
            x5.transpose(5, 3, 0, 4, 1, 2, 6)          # [k,j,b,gm,qh,qp,d]
        ).reshape(128, NSLAB * D)
        m = dict(shared)
        m["xt"] = xt
        m["xpp"] = xpp
        in_maps.append(m)
    return in_maps


def kernel(x, Wq, Wk, Wv, Wo, bo):
    """Full-input entry point: shards batch over 8 cores, returns full output."""
    from concourse.bass_utils import run_bass_kernel_spmd

    nc = build()
    in_maps = host_inputs(x, Wq, Wk, Wv, Wo, bo)
    res = run_bass_kernel_spmd(nc, in_maps, list(range(N_CORES)))
    out = np.concatenate([np.asarray(res.results[c]["y"], dtype=np.float32)
                          for c in range(N_CORES)], axis=0)
    return out.reshape(B_GLOB, RB, E)


# revision 16
# speedup vs baseline: 1.0382x; 1.0382x over previous
"""Trainium2 Bass kernel for block-local MultiHeadAttention + output projection.

Reference computation (per batch b):
  Q = x @ Wq.T ; K = x @ Wk.T ; V = x @ Wv.T          x: [B, S=8192, 64]
  reshape to [B, G=512, H=16, 64] (groups of 16 consecutive tokens)
  E[g,h,k] = Q[g,h,:] . K[g,k,:]                      (16x16 block-diag attention)
  A = softmax(E / 32, axis=k)
  O[g,h,:] = sum_k A[g,h,k] V[g,k,:]
  out2[b, r, gm*64+d] = O[g=(gq,gm), h, d]  with r = h*32+gq
  y = out2 @ Wo.T + bo                                y: [B, 512, 1024]

Kernel strategy (data-parallel over batch, 4 batches/core on 8 cores):
  - host prep: M^T = Wk^T Wq (so E[h,k] = X_h . Z_k with Z = X M^T, skipping
    Q,K), WoV^T = (Wo @ blockdiag(Wv))^T in fc-ready chunk layout (skips V),
    x cast to bf16 and pre-marshalled into the two SBUF layouts the PE
    consumes (XT feature-major, XPP slab-token-major), plus constants
  - ZT = M X^T on device (row-tiled concurrent matmul pairs)
  - per gm-PAIR (c = gm//2): 8 E^T matmuls (row-tiled by q-parity into
    parity-split banks of one contiguous [128,1024] eps tile), ONE exp,
    ONE mask mul, then per gm 4 U^T matmuls + 1 den ones-matmul into a
    shared [128,1024] ud tile (U cols 0-511 q-order, den cols 512+ aex-order)
  - normalization: reciprocal_approx_fast(den) once per pair; the mul's
    in1 AP permutes aex->q order on the fly
  - fc: y-tile = (out2^T-tile stationary) @ WoV^T streaming; bias added via
    tensor_add during the PSUM eviction; y stored bf16, host casts f32
"""

import numpy as np
from contextlib import ExitStack

import concourse.bass as bass
import concourse.bacc as bacc
import concourse.mybir as mybir
import concourse.tile as tile

N_CORES = 8
B_GLOB = 32
B_LOC = B_GLOB // N_CORES   # 4 batches per core
SB = 8192                   # tokens per batch
D = 64                      # head dim
NG = 16                     # gm values (heads)
NQ = 4                      # gq octs per batch
NJ = 8                      # groups per slab
NH = 16                     # tokens per group
E = 1024
RB = 512                    # out2 rows per batch
NSLAB = B_LOC * NG * NQ     # 256 slabs per core
TOK = B_LOC * SB            # 32768 tokens per core

BF = mybir.dt.bfloat16
F32 = mybir.dt.float32
AF = mybir.ActivationFunctionType


def slab_xt_ap(T, b, gm, q):
    """[64@(q%2), 128] contiguous view of slab (b,gm,q) in XT/ZT layout:
    col = (sidx//2)*128 + k*8 + j, rows (q%2)*64 + d."""
    sidx = (b * 16 + gm) * 4 + q
    half = (sidx % 2) * 64
    pair = sidx // 2
    return T[half:half + 64, pair * 128:(pair + 1) * 128]


def emit_body(ctx, tc, ins, outs, dbg, stage=99):
    nc = tc.nc
    xt, xpp, mt, wovt, mask2, ones64, bias128 = ins
    y = outs["y"]

    # ---------------- persistent tensors ----------------
    pp = ctx.enter_context(tc.tile_pool(name="persist", bufs=1))
    XT = pp.tile([128, 8 * NH * 128], BF, tag="XT")     # [(q%2)*64+d, (bQ|k|j,gm)]
    ZT = pp.tile([128, 8 * NH * 128], BF, tag="ZT")
    XPP = pp.tile([128, NSLAB * D], BF, tag="XPP")      # [k*8+j, (sidx|d)]
    WOVT = pp.tile([128, 8 * E], BF, tag="WOVT")        # WoV^T chunks
    OUT2T = pp.tile([128, B_LOC * 8 * RB], BF, tag="OUT2T")
    MASK2 = pp.tile([128, 1024], BF, tag="MASK2")       # kron(ones16, eye8) x8
    ONES64 = pp.tile([128, D], BF, tag="ONES64")
    MT = pp.tile([128, D], BF, tag="MT")                # M^T dup on both halves
    BIAS = pp.tile([128, E], F32, tag="BIAS")           # bias replicated 128 rows

    # ---------------- input loads, ordered by first use --------
    # queues are FIFO per engine: batch-0 x chunks and the small consts go
    # first; WOVT/BIAS (only needed by fc, ~60us in) go last
    if stage < 1:
        return
    CB = 8 * NH * 128 // B_LOC      # XT/XPP cols per batch (4096)
    # batch 0 lands in fine-grained chunks so ZT/attention start ASAP
    nc.sync.dma_start(XT[:, 0:1024], xt[:, 0:1024])
    nc.sync.dma_start(MT[:], mt)
    nc.sync.dma_start(XPP[:, 0:1024], xpp[:, 0:1024])
    nc.sync.dma_start(MASK2[:], mask2)
    nc.sync.dma_start(ONES64[:], ones64)
    for s in range(1, 4):
        nc.sync.dma_start(XT[:, s * 1024:(s + 1) * 1024],
                          xt[:, s * 1024:(s + 1) * 1024])
        nc.sync.dma_start(XPP[:, s * 1024:(s + 1) * 1024],
                          xpp[:, s * 1024:(s + 1) * 1024])
    for b in range(1, B_LOC):
        nc.sync.dma_start(XT[:, b * CB:(b + 1) * CB], xt[:, b * CB:(b + 1) * CB])
        nc.sync.dma_start(XPP[:, b * CB:(b + 1) * CB], xpp[:, b * CB:(b + 1) * CB])
    nc.sync.dma_start(BIAS[:], bias128)
    nc.sync.dma_start(WOVT[:], wovt)

    if stage < 2:
        return
    big_pool = ctx.enter_context(tc.tile_pool(name="bigps", bufs=2, space="PSUM"))
    eps_pool = fc_pool = big_pool
    ud_pool = ctx.enter_context(tc.tile_pool(name="ud", bufs=2, space="PSUM"))
    zt_pool = ud_pool               # ZT phase borrows the ud slots
    aex_pool = ctx.enter_context(tc.tile_pool(name="aex", bufs=4))
    am_pool = ctx.enter_context(tc.tile_pool(name="am", bufs=4))
    rden_pool = ctx.enter_context(tc.tile_pool(name="rden", bufs=3))
    fout_pool = ctx.enter_context(tc.tile_pool(name="fout", bufs=2))

    def emit_zt(b, rb):
        """One 1024-col ZT chunk for batch b (borrows a ud-pool slot)."""
        r = b * (CB // 1024) + rb
        zfull = zt_pool.tile([128, 1024], F32, tag="ud")
        for hz in range(2):
            zps = zfull[:, hz * 512:(hz + 1) * 512]
            cl = (r * 2 + hz) * 512
            nc.tensor.matmul(zps[0:64, :], MT[0:64, :],
                             XT[0:64, cl:cl + 512], start=True, stop=True)
            nc.tensor.matmul(zps[64:128, :], MT[64:128, :],
                             XT[64:128, cl:cl + 512], start=True, stop=True,
                             tile_position=(64, 64))
        nc.any.tensor_copy(ZT[:, r * 1024:(r + 1) * 1024], zfull[:])

    for b in range(B_LOC):
        # ---------------- ZT = M X^T for this batch ----------------
        if stage < 3:
            continue
        for rb in range(CB // 1024):
            emit_zt(b, rb)

        # ---------------- attention, per gm pair ----------------
        if stage < 4:
            continue
        for c in range(8):
            # E^T matmuls for both gms of the pair, row-tiled by q-parity;
            # eps col = (q%2)*512 + (gm%2)*256 + (q//2)*128  (parity-split banks)
            eps = eps_pool.tile([128, 1024], F32, tag="bigps")
            for gmh in range(2):
                gm = c * 2 + gmh
                for q in range(NQ):
                    col = (q % 2) * 512 + gmh * 256 + (q // 2) * 128
                    nc.tensor.matmul(
                        eps[:, col:col + 128],
                        slab_xt_ap(ZT, b, gm, q),
                        slab_xt_ap(XT, b, gm, q),
                        start=True, stop=True,
                        tile_position=((q % 2) * 64, 0),
                    )
            if stage < 4.2:
                continue
            aex = aex_pool.tile([128, 1024], BF, tag="aex")
            nc.scalar.activation(aex[:], eps[:], AF.Exp, scale=1.0 / 32.0)
            if stage < 4.4:
                continue
            am = am_pool.tile([128, 1024], BF, tag="am")
            nc.vector.tensor_mul(am[:], aex[:], MASK2[:])
            if stage < 4.6:
                continue
            ud = ud_pool.tile([128, 1024], F32, tag="ud")
            amr = am[:].rearrange("p (par gmh2 qh hj) -> gmh2 p par qh hj",
                                  par=2, gmh2=2, qh=2)
            for gmh in range(2):
                gm = c * 2 + gmh
                pb = gmh * 64
                for q in range(NQ):
                    sidx = (b * 16 + gm) * 4 + q
                    acol = (q % 2) * 512 + gmh * 256 + (q // 2) * 128
                    nc.tensor.matmul(ud[pb:pb + 64, q * 128:(q + 1) * 128],
                                     XPP[:, sidx * D:(sidx + 1) * D],
                                     am[:, acol:acol + 128],
                                     start=True, stop=True, tile_position=(0, pb))
                nc.tensor.matmul(ud[pb:pb + 64, 512:1024], ONES64[:], amr[gmh],
                                 start=True, stop=True, tile_position=(0, pb))
            if stage < 4.8:
                continue
            # rden in aex order [par qh hj]; the mul's in1 AP maps to q order
            rden = rden_pool.tile([128, 512], F32, tag="rden")
            nc.vector.reciprocal_approx_fast(rden[:], ud[:, 512:1024])
            rden_q = rden[:].rearrange("p (Y X hj) -> p X Y hj",
                                       Y=2, X=2, hj=128)
            sec = (b * 8 + c) * 512
            out_ap = OUT2T[:, sec:sec + 512].rearrange(
                "p (h q2 j) -> p q2 h j", h=NH, q2=NQ, j=NJ)
            nc.vector.tensor_mul(out_ap, ud[:, 0:512], rden_q)

        # ---------------- fc for this batch ----------------
        if stage < 6:
            continue
        for rt in range(4):
            fo = fout_pool.tile([128, E], BF, tag="fout")
            for halfe in range(2):
                fps_full = fc_pool.tile([128, 1024], F32, tag="bigps")
                fps = fps_full[:, 0:512]
                for cc in range(8):
                    sec = (b * 8 + cc) * 512
                    nc.tensor.matmul(
                        fps[:],
                        OUT2T[:, sec + rt * 128: sec + (rt + 1) * 128],
                        WOVT[:, cc * E + halfe * 512: cc * E + halfe * 512 + 512],
                        start=(cc == 0), stop=(cc == 7),
                    )
                nc.any.tensor_add(fo[:, halfe * 512:(halfe + 1) * 512],
                                  fps[:],
                                  BIAS[:, halfe * 512:(halfe + 1) * 512])
            row = b * RB + rt * 128
            nc.sync.dma_start(y[row:row + 128, :], fo[:])

    # ---------------- debug dumps ----------------
    for name, T in (("xt", XT), ("zt", ZT), ("xpp", XPP), ("out2t", OUT2T)):
        if name in dbg:
            nc.sync.dma_start(dbg[name], T[:])


def build(reps=1, debug=(), stage=99):
    nc = bacc.Bacc("TRN2", target_bir_lowering=False, debug=False,
                   num_devices=N_CORES)
    xt = nc.dram_tensor("xt", [128, 8 * NH * 128], BF, kind="ExternalInput").ap()
    xpp = nc.dram_tensor("xpp", [128, NSLAB * D], BF, kind="ExternalInput").ap()
    mt = nc.dram_tensor("mt", [128, D], BF, kind="ExternalInput").ap()
    wovt = nc.dram_tensor("wovt", [128, 8 * E], BF, kind="ExternalInput").ap()
    mask2 = nc.dram_tensor("mask2", [128, 1024], BF, kind="ExternalInput").ap()
    ones64 = nc.dram_tensor("ones64", [128, D], BF, kind="ExternalInput").ap()
    bias128 = nc.dram_tensor("bias128", [128, E], F32, kind="ExternalInput").ap()
    y = nc.dram_tensor("y", [B_LOC * RB, E], BF, kind="ExternalOutput").ap()
    dbg = {}
    for name, shape, dt in [
        ("xt", [128, 8 * NH * 128], BF),
        ("zt", [128, 8 * NH * 128], BF),
        ("xpp", [128, NSLAB * D], BF),
        ("out2t", [128, B_LOC * 8 * RB], BF),
    ]:
        if name in debug:
            dbg[name] = nc.dram_tensor(name, shape, dt, kind="ExternalOutput").ap()

    ins = (xt, xpp, mt, wovt, mask2, ones64, bias128)
    outs = {"y": y}
    with tile.TileContext(nc) as tc:
        with ExitStack() as ctx:
            if reps > 1:
                with tc.For_i(0, reps, 1):
                    emit_body(ctx, tc, ins, outs, dbg, stage=stage)
            else:
                emit_body(ctx, tc, ins, outs, dbg, stage=stage)
    nc.compile()
    return nc


def host_inputs(x, Wq, Wk, Wv, Wo, bo):
    """Host-side weight prep + x marshalling shared by kernel() and tests."""
    import ml_dtypes
    bf16 = ml_dtypes.bfloat16
    x = np.asarray(x, np.float32)
    Wq = np.asarray(Wq, np.float32)
    Wk = np.asarray(Wk, np.float32)
    Wv = np.asarray(Wv, np.float32)
    Wo = np.asarray(Wo, np.float32)
    bo = np.asarray(bo, np.float32)

    MTh = Wk.T @ Wq                                    # M^T, M = Wq.T @ Wk
    mt = np.concatenate([MTh, MTh], axis=0).astype(bf16)          # [128, 64]
    # WoV[e, g*64+d] = sum_v Wo[e, g*64+v] Wv[v, d]
    WoV = np.matmul(Wo.reshape(E, NG, D), Wv).reshape(E, E)
    WoVT = np.ascontiguousarray(WoV.T)                 # [feature, e_out]
    wovt = np.ascontiguousarray(
        WoVT.reshape(8, 128, E).transpose(1, 0, 2).reshape(128, 8 * E)
    ).astype(bf16)
    blk = np.kron(np.ones((16, 16), np.float32), np.eye(8, dtype=np.float32))
    mask2 = np.tile(blk, (1, 8)).astype(bf16)
    ones64 = np.ones((128, D), dtype=bf16)
    bias128 = np.ascontiguousarray(np.tile(bo[None, :], (128, 1)),
                                   dtype=np.float32)
    xbf = x.astype(bf16)
    shared = {"mt": mt, "wovt": wovt, "mask2": mask2, "ones64": ones64,
              "bias128": bias128}
    in_maps = []
    for core in range(N_CORES):
        xs = xbf[core * B_LOC:(core + 1) * B_LOC]
        # token t = ((q*8+j)*16+gm)*16+k, q = qh*2+qp
        x5 = xs.reshape(B_LOC, 2, 2, NJ, NG, NH, D)   # [b,qh,qp,j,gm,k,d]
        xt = np.ascontiguousarray(
            x5.transpose(2, 6, 0, 4, 1, 5, 3)          # [qp,d,b,gm,qh,k,j]
        ).reshape(128, 8 * NH * 128)
        xpp = np.ascontiguousarray(
            x5.transpose(5, 3, 0, 4, 1, 2, 6)          # [k,j,b,gm,qh,qp,d]
        ).reshape(128, NSLAB * D)
        m = dict(shared)
        m["xt"] = xt
        m["xpp"] = xpp
        in_maps.append(m)
    return in_maps


def kernel(x, Wq, Wk, Wv, Wo, bo):
    """Full-input entry point: shards batch over 8 cores, returns full output."""
    from concourse.bass_utils import run_bass_kernel_spmd

    nc = build()
    in_maps = host_inputs(x, Wq, Wk, Wv, Wo, bo)
    res = run_bass_kernel_spmd(nc, in_maps, list(range(N_CORES)))
    out = np.concatenate([np.asarray(res.results[c]["y"], dtype=np.float32)
                          for c in range(N_CORES)], axis=0)
    return out.reshape(B_GLOB, RB, E)
